# revision 41
# baseline (speedup 1.0000x reference)
"""Causal self-attention (B=4, T=2048, D=1024, H=16) on 8 NeuronCores - v2.

Sharding: core c handles batch b=c//2 and head-group hg=c%2 (8 of 16 heads,
processed as 4 head pairs). Column-parallel Wq/Wk/Wv (512 cols), row-parallel
Wo (512 rows). Host sums the two partial outputs per batch and adds bo.

All-bf16 PE compute, fp32 PSUM accumulation. x^T and every weight is loaded
into SBUF once and stays resident (no per-pair re-reads). The k-side bias is
dropped: (q+bq).(k+bk) differs from (q+bq).k by a per-query constant, which
softmax cancels exactly. The q bias is folded into the projection as a K=1
rank-1 matmul.

Attention per (pair, query-block): score matmuls row-packed 2 heads via
tile_position; ONE exp per k-tile covering both heads ([128, 2, 512] PSUM
tile spanning 2 banks); causal masking on diagonal k-tiles via a post-exp
multiply with a 128x128 triangular 0/1 mask (only the one partial subtile);
PV with a ones column producing sumexp. Normalization: sumexp rows gathered
by DVE copies into [8, 512], one batched reciprocal per pair, broadcast to
[128, 512] via a bf16 K=8 selection matmul, applied with one DVE multiply.

Q/K projections of pair c+1 are emitted after attention of pair c so the Tile
scheduler fills PE bubbles with projection matmuls (keeps HAM at 2.4 GHz).
"""

import os
from contextlib import ExitStack

import ml_dtypes
import numpy as np

import concourse.bacc as bacc
import concourse.mybir as mybir
import concourse.tile as tile
from concourse.bass_utils import run_bass_kernel_spmd

B, T, D, H, DK = 4, 2048, 1024, 16, 64
HL = 8  # heads per core
CD = HL * DK  # 512 local channels
NP = 128
QB = 512
NDC = D // NP  # 8 din chunks
NTT = T // NP  # 16 t-tiles
NTB = T // QB  # 4 t-blocks
NPAIR = HL // 2  # 4 head pairs
F32 = mybir.dt.float32
BF16 = mybir.dt.bfloat16
Exp = mybir.ActivationFunctionType.Exp

_CACHE: dict = {}


def _build_nc():
    nc = bacc.Bacc("TRN2", target_bir_lowering=False, debug=False)
    # xt pre-chunked host-side: [tb, d, p, q] so each DMA is contiguous
    xt = nc.dram_tensor("xt", [NTB, NDC, NP, QB], BF16, kind="ExternalInput")
    wq = nc.dram_tensor("wq", [D, CD], BF16, kind="ExternalInput")
    wk = nc.dram_tensor("wk", [D, CD], BF16, kind="ExternalInput")
    wv = nc.dram_tensor("wv", [D, CD], BF16, kind="ExternalInput")
    wo = nc.dram_tensor("wo", [CD, D], BF16, kind="ExternalInput")
    bqr = nc.dram_tensor("bqr", [NP, NPAIR], F32, kind="ExternalInput")
    bvr = nc.dram_tensor("bvr", [NP, CD], BF16, kind="ExternalInput")
    tri = nc.dram_tensor("tri", [NP, 2, NP], BF16, kind="ExternalInput")
    y = nc.dram_tensor("y", [T, D], F32, kind="ExternalOutput")

    with tile.TileContext(nc) as tc, ExitStack() as ctx:
        _body(nc, tc, ctx, xt, wq, wk, wv, wo, bqr, bvr, tri, y)
    nc.compile()
    return nc


def _body(nc, tc, ctx, xt, wq, wk, wv, wo, bqr, bvr, tri, y):
    const = ctx.enter_context(tc.tile_pool(name="const", bufs=1))
    wpool = ctx.enter_context(tc.tile_pool(name="w", bufs=1))
    xpool = ctx.enter_context(tc.tile_pool(name="x", bufs=1))
    vpool = ctx.enter_context(tc.tile_pool(name="v", bufs=1))
    oatp = ctx.enter_context(tc.tile_pool(name="oat", bufs=1))
    qkp = ctx.enter_context(tc.tile_pool(name="qk", bufs=2))
    etp = ctx.enter_context(tc.tile_pool(name="et", bufs=4))
    zp = ctx.enter_context(tc.tile_pool(name="z", bufs=2))
    ystp = ctx.enter_context(tc.tile_pool(name="yst", bufs=4))
    # PSUM: proj 2 banks + score 2x2 banks + pv 2 banks = 8
    projps = ctx.enter_context(tc.tile_pool(name="projps", bufs=2, space="PSUM"))
    scoreps = ctx.enter_context(tc.tile_pool(name="scoreps", bufs=2, space="PSUM"))
    pvps = ctx.enter_context(tc.tile_pool(name="pvps", bufs=1, space="PSUM"))

    # constants
    bq_sb = const.tile([NP, NPAIR], F32, tag="bq")
    nc.sync.dma_start(bq_sb[:], bqr[:])
    bv_sb = const.tile([NP, CD], BF16, tag="bv")
    nc.sync.dma_start(bv_sb[:], bvr[:])
    tri_sb = const.tile([NP, 2, NP], BF16, tag="tri")
    nc.sync.dma_start(tri_sb[:], tri[:])
    # warm up the exp table set early (one-time ~2.7us load overlaps V phase)
    warm = const.tile([1, 2], F32, tag="warm")
    nc.vector.memset(warm[:], 0.0)
    nc.scalar.activation(warm[:], warm[:], Exp)

    # resident x^T and weights; order matters: the V phase needs wv + the
    # first xt block before any matmul can start.
    xt_sb = xpool.tile([NP, NDC, T], BF16, tag="xt")
    wv_sb = wpool.tile([NP, NDC, CD], BF16, tag="wv")
    wq_sb = wpool.tile([NP, NDC, CD], BF16, tag="wq")
    wk_sb = wpool.tile([NP, NDC, CD], BF16, tag="wk")
    wo_sb = wpool.tile([NP, NPAIR, D], BF16, tag="wo")
    for d in range(NDC):
        nc.sync.dma_start(wv_sb[:, d, :], wv[d * NP : (d + 1) * NP, :])
    for d in range(NDC):
        nc.sync.dma_start(xt_sb[:, d, 0:QB], xt[0, d, :, :])
    for d in range(NDC):
        nc.sync.dma_start(wq_sb[:, d, :], wq[d * NP : (d + 1) * NP, :])
        nc.sync.dma_start(wk_sb[:, d, :], wk[d * NP : (d + 1) * NP, :])
    for tb in range(1, NTB):
        for d in range(NDC):
            nc.sync.dma_start(
                xt_sb[:, d, tb * QB : (tb + 1) * QB], xt[tb, d, :, :]
            )
    for cc in range(NPAIR):
        nc.sync.dma_start(wo_sb[:, cc, :], wo[cc * NP : (cc + 1) * NP, :])

    # ---- V phase: V[t, 8 heads x (64 dv + ones cols)] ----
    v_sb = [
        vpool.tile([NP, HL, DK + 1], BF16, tag=f"v{tt}", name=f"v{tt}")
        for tt in range(NTT)
    ]

    def vphase(tb):
        for i in range(QB // NP):
            tt = tb * (QB // NP) + i
            ps = projps.tile([NP, CD], F32, tag="proj")
            for d in range(NDC):
                nc.tensor.matmul(
                    ps[:],
                    xt_sb[:, d, tt * NP : (tt + 1) * NP],
                    wv_sb[:, d, :],
                    start=(d == 0),
                    stop=(d == NDC - 1),
                )
            vt = v_sb[tt]
            nc.vector.memset(vt[:, :, DK : DK + 1], 1.0)
            nc.vector.tensor_add(
                vt[:, :, 0:DK],
                ps.rearrange("p (h k) -> p h k", h=HL),
                bv_sb.rearrange("p (h k) -> p h k", h=HL),
            )

    oat = [oatp.tile([NP, T], BF16, tag=f"oat{c}", name=f"oat{c}") for c in range(NPAIR)]

    def proj(c):
        qt = qkp.tile([NP, T], BF16, tag="qt", name=f"qt{c}")
        kt = qkp.tile([NP, T], BF16, tag="kt", name=f"kt{c}")
        for tb in range(NTB):
            sl = slice(tb * QB, (tb + 1) * QB)
            psq = projps.tile([NP, QB], F32, tag="proj")
            for d in range(NDC):
                nc.tensor.matmul(
                    psq[:], wq_sb[:, d, c * NP : (c + 1) * NP], xt_sb[:, d, sl],
                    start=(d == 0), stop=(d == NDC - 1),
                )
            nc.vector.tensor_scalar_add(qt[:, sl], psq[:], bq_sb[:, c : c + 1])
            psk = projps.tile([NP, QB], F32, tag="proj")
            for d in range(NDC):
                nc.tensor.matmul(
                    psk[:], wk_sb[:, d, c * NP : (c + 1) * NP], xt_sb[:, d, sl],
                    start=(d == 0), stop=(d == NDC - 1),
                )
            nc.vector.tensor_copy(kt[:, sl], psk[:])
        return qt, kt

    def attn_qb(c, qt, kt, qb):
            nkt = 4 * qb + 4
            pv = pvps.tile([DK + 1, 2, QB], F32, tag="pv")
            for kti in range(nkt):
                di = kti - 4 * qb
                o = max(di, 0) * NP
                sps = scoreps.tile([NP, 2, QB], F32, tag="s")
                for h in range(2):
                    nc.tensor.matmul(
                        sps[:, h, o:QB],
                        kt[64 * h : 64 * h + 64, kti * NP : (kti + 1) * NP],
                        qt[64 * h : 64 * h + 64, qb * QB + o : (qb + 1) * QB],
                        start=True, stop=True,
                        tile_position=(64 * h, 0),
                    )
                et = etp.tile([NP, 2, QB], BF16, tag="et")
                nc.scalar.activation(et[:, :, o:QB], sps[:, :, o:QB], Exp, scale=0.125)
                if di >= 0:
                    nc.vector.tensor_mul(
                        et[:, :, o : o + NP], et[:, :, o : o + NP], tri_sb[:]
                    )
                for h in range(2):
                    nc.tensor.matmul(
                        pv[:, h, o:QB],
                        v_sb[kti][:, 2 * c + h, :],
                        et[:, h, o:QB],
                        start=(kti == 0), stop=(kti == nkt - 1),
                    )
            # per-qb normalize: stage sumexp rows, DMA-scatter to [8, 128],
            # batched reciprocal, K=8 selection matmuls broadcast 1/Z, apply.
            zsb = zp.tile([1, 2, QB], F32, tag="zsb")
            with tc.high_priority():
                nc.vector.tensor_copy(zsb[0:1, :, :], pv[DK : DK + 1, :, :])
                for h in range(2):
                    nc.vector.tensor_copy(
                        oat[c][64 * h : 64 * h + 64, qb * QB : (qb + 1) * QB],
                        pv[0:DK, h, :],
                    )
            za = zp.tile([8, NP], F32, tag="za")
            for h in range(2):
                nc.sync.dma_start(za[4 * h : 4 * h + 4, :], zsb[0:1, h, :])
            zr = zp.tile([8, NP], BF16, tag="zr")
            with nc.allow_low_precision(reason="1/sumexp bf16 is intentional"):
                nc.vector.reciprocal(zr[:], za[:])
            zrl = zp.tile([1, 2, QB], BF16, tag="zrl")
            for h in range(2):
                nc.sync.dma_start(zrl[0:1, h, :], zr[4 * h : 4 * h + 4, :])
            bcz = zp.tile([NP, QB], BF16, tag="bcz")
            tmph = zp.tile([DK, QB], BF16, tag="tmph")
            nc.gpsimd.partition_broadcast(bcz[0:DK, :], zrl[0:1, 0, :])
            nc.gpsimd.partition_broadcast(tmph[0:DK, :], zrl[0:1, 1, :])
            nc.vector.tensor_copy(bcz[DK:NP, :], tmph[0:DK, :])
            nc.vector.tensor_mul(
                oat[c][:, qb * QB : (qb + 1) * QB],
                oat[c][:, qb * QB : (qb + 1) * QB],
                bcz[:],
            )

    for tb in range(NTB):
        vphase(tb)
    qts = proj(0)
    for c in range(NPAIR):
        for qb in range(NTB):
            attn_qb(c, qts[0], qts[1], qb)
        if c + 1 < NPAIR:
            qts = proj(c + 1)

    # ---- final projection: y[t, dout] = oat.T @ Wo ----
    for tt in range(NTT):
        pss = [
            projps.tile([NP, QB], F32, tag="proj", name=f"yps{dh}") for dh in range(2)
        ]
        for cc in range(NPAIR):
            for dh in range(2):
                nc.tensor.matmul(
                    pss[dh][:],
                    oat[cc][:, tt * NP : (tt + 1) * NP],
                    wo_sb[:, cc, dh * QB : (dh + 1) * QB],
                    start=(cc == 0), stop=(cc == NPAIR - 1),
                )
        for dh in range(2):
            ys = ystp.tile([NP, QB], F32, tag="ys")
            nc.vector.tensor_copy(ys[:], pss[dh][:])
            nc.sync.dma_start(
                y[tt * NP : (tt + 1) * NP, dh * QB : (dh + 1) * QB], ys[:]
            )


def _install_ntff_hook_shim():
    """The agent image's antenv lacks axon_hooks, so trace=True under axon
    degrades. Provide the missing module and register the ctypes NTFF hook
    from trn_agent_boot. Best-effort: failures just mean no trace."""
    try:
        import sys
        import types

        if "antenv.axon_hooks" not in sys.modules:
            mod = types.ModuleType("antenv.axon_hooks")
            mod._hook = None
            mod.set_axon_ntff_profile_hook = lambda h: setattr(mod, "_hook", h)
            mod.get_axon_ntff_profile_hook = lambda: mod._hook
            sys.modules["antenv.axon_hooks"] = mod
            import antenv

            antenv.axon_hooks = mod
        from antenv.axon_hooks import (
            get_axon_ntff_profile_hook,
            set_axon_ntff_profile_hook,
        )

        if get_axon_ntff_profile_hook() is None:
            from trn_agent_boot.trn_boot import _ntff_profile_via_ctypes

            hook = _ntff_profile_via_ctypes("/opt/axon/libaxon_pjrt.so")
            if hook is not None:
                set_axon_ntff_profile_hook(hook)
    except Exception as e:  # noqa: BLE001
        print(f"ntff hook shim failed ({e}); running without trace")


def _bf(a: np.ndarray) -> np.ndarray:
    return np.ascontiguousarray(a, dtype=np.float32).astype(ml_dtypes.bfloat16)


def _make_tri() -> np.ndarray:
    t = (np.arange(NP)[None, :] >= np.arange(NP)[:, None]).astype(np.float32)
    return _bf(np.stack([t, t], axis=1))


def kernel(x, Wq, bq, Wk, bk, Wv, bv, Wo, bo):
    x = np.ascontiguousarray(np.asarray(x, dtype=np.float32))
    Wq, bq = np.asarray(Wq, np.float32), np.asarray(bq, np.float32)
    Wk = np.asarray(Wk, np.float32)
    Wv, bv = np.asarray(Wv, np.float32), np.asarray(bv, np.float32)
    Wo, bo = np.asarray(Wo, np.float32), np.asarray(bo, np.float32)

    if "nc" not in _CACHE:
        _CACHE["nc"] = _build_nc()
    nc = _CACHE["nc"]

    tri = _make_tri()
    in_maps = []
    for core in range(8):
        b, hg = core // 2, core % 2
        cs = slice(hg * CD, (hg + 1) * CD)
        in_maps.append(
            {
                "xt": _bf(
                    x[b].T.reshape(NDC, NP, NTB, QB).transpose(2, 0, 1, 3)
                ),
                "wq": _bf(Wq[:, cs]),
                "wk": _bf(Wk[:, cs]),
                "wv": _bf(Wv[:, cs]),
                "wo": _bf(Wo[cs, :]),
                "bqr": np.ascontiguousarray(
                    bq[cs].reshape(NPAIR, NP).T, dtype=np.float32
                ),
                "bvr": _bf(np.tile(bv[cs].reshape(1, CD), (NP, 1))),
                "tri": tri,
            }
        )

    trace = bool(os.environ.get("KERNEL_TRACE"))
    if trace:
        _install_ntff_hook_shim()
    res = run_bass_kernel_spmd(
        nc, in_maps, core_ids=list(range(8)), trace=trace
    )
    _CACHE["last_results"] = res

    out = np.empty((B, T, D), dtype=np.float32)
    for b in range(B):
        out[b] = res.results[2 * b]["y"] + res.results[2 * b + 1]["y"] + bo
    return out
